# revision 9
# baseline (speedup 1.0000x reference)
"""MoE layer (E=8 experts, top-2, D=1024, H=4096, N=4096 tokens) on 8 TRN2
NeuronCores.

Strategy: data-parallel over tokens (512 tokens/core), experts replicated.
Each core computes the full MoE for its token shard entirely on device:
gate matmul + softmax-top2 routing, then the dense expert sum
   out = sum_e comb[:, e] * (gelu(x @ w1[e] + b1[e]) @ w2[e] + b2[e])
with comb[t, e] = 0 for experts outside token t's top-2 (so only the top-2
terms contribute, exactly matching the reference formulation).

All activations are kept transposed on device (feature dim on partitions,
tokens on the free axis) so both expert matmuls consume weights in their
natural layout and chain without transposes:
   hT[h, t]  = sum_d w1[d, h] * xT[d, t]        (lhsT = w1 tile, rhs = xT)
   outT[d, t] = sum_h w2[h, d] * geluT[h, t]     (lhsT = w2 tile, rhs = geluT)
Host does the pure layout work: shard + transpose x, pre-tile the weights,
transpose the returned outT shards back.

Matmuls run in fp32r (TF32-like, 1 PE cycle/row at N=512 -> ~78 TFLOP/s/core)
with fp32 PSUM accumulation.
"""

import numpy as np

import concourse.bass as bass  # noqa: F401  (bass types used via tile/bacc)
import concourse.mybir as mybir
import concourse.tile as tile
from concourse import bacc, bass_utils

F32 = mybir.dt.float32
F32R = mybir.dt.float32r
AFT = mybir.ActivationFunctionType
ALU = mybir.AluOpType

E = 8          # experts
D = 1024       # model dim
H = 4096       # expert hidden dim
P = 128        # partitions
NCORES = 8
NTOK = 4096    # total tokens (B*T = 2*2048)
T = NTOK // NCORES   # 512 tokens per core
KD = D // P    # 8 contraction chunks of D
NH = H // P    # 32 h tiles
ND = D // P    # 8 d tiles
TT = T // P    # 4 token tiles of 128

_NC = None  # cached compiled module


def _build():
    nc = bacc.Bacc("TRN2", target_bir_lowering=False, debug=False,
                   num_devices=NCORES)
    xT = nc.dram_tensor("xT", [D, T], F32R, kind="ExternalInput").ap()
    xTf = nc.dram_tensor("xTf", [D, T], F32, kind="ExternalInput").ap()
    gwt = nc.dram_tensor("gwt", [P, KD, E], F32, kind="ExternalInput").ap()
    gb = nc.dram_tensor("gb", [1, E], F32, kind="ExternalInput").ap()
    w1t = nc.dram_tensor("w1t", [E, NH, P, KD, P], F32R,
                         kind="ExternalInput").ap()
    b1t = nc.dram_tensor("b1t", [E, P, NH], F32, kind="ExternalInput").ap()
    w2t = nc.dram_tensor("w2t", [E, ND, P, NH, P], F32R,
                         kind="ExternalInput").ap()
    b2n = nc.dram_tensor("b2n", [E, D], F32R, kind="ExternalInput").ap()
    ones = nc.dram_tensor("ones", [1, P], F32, kind="ExternalInput").ap()
    ident = nc.dram_tensor("ident", [P, P], F32, kind="ExternalInput").ap()
    outT = nc.dram_tensor("outT", [D, T], F32, kind="ExternalOutput").ap()

    with tile.TileContext(nc) as tc:
        with (
            tc.tile_pool(name="const", bufs=1) as cpool,
            tc.tile_pool(name="w1p", bufs=3) as w1p,
            tc.tile_pool(name="w2p", bufs=2) as w2p,
            tc.tile_pool(name="bias", bufs=2) as biasp,
            tc.tile_pool(name="rt", bufs=2) as rt,
            tc.tile_pool(name="xfp", bufs=3) as xfp,
            tc.tile_pool(name="gtmp", bufs=4) as gtmp,
            tc.tile_pool(name="psh", bufs=2, space="PSUM") as psh,
            tc.tile_pool(name="pso", bufs=2, space="PSUM") as pso,
            tc.tile_pool(name="psr", bufs=2, space="PSUM") as psr,
        ):
            # ---- persistent SBUF ----
            xTt = cpool.tile([P, KD, T], F32R)
            for kd in range(KD):
                nc.sync.dma_start(xTt[:, kd, :], xT[kd * P:(kd + 1) * P, :])
            gwt_s = cpool.tile([P, KD, E], F32)
            nc.sync.dma_start(gwt_s[:], gwt[:])
            gb_s = cpool.tile([1, E], F32)
            nc.sync.dma_start(gb_s[:], gb[:])
            ones_s = cpool.tile([1, P], F32)
            nc.sync.dma_start(ones_s[:], ones[:])
            id_s = cpool.tile([P, P], F32)
            nc.sync.dma_start(id_s[:], ident[:])
            b2_s = cpool.tile([E, D], F32R)
            nc.sync.dma_start(b2_s[:], b2n[:])
            combT8 = cpool.tile([E, T], F32)    # comb.T on partitions 0..7
            combT8r = cpool.tile([E, T], F32R)  # f32r copy for the b2 matmul
            comb_b = cpool.tile([P, E, T], F32)
            accT = cpool.tile([P, ND, T], F32)
            geluT = cpool.tile([P, NH, T], F32R)

            # ---- routing: logits -> top-2 -> comb weights ----
            for tt in range(TT):
                tok = slice(tt * P, (tt + 1) * P)
                ps_l = psr.tile([P, E], F32, tag="psr")
                for kd in range(KD):
                    xf = xfp.tile([P, P], F32, tag="xf")
                    nc.sync.dma_start(xf[:], xTf[kd * P:(kd + 1) * P, tok])
                    nc.tensor.matmul(ps_l[:], xf[:], gwt_s[:, kd, :],
                                     start=(kd == 0), stop=False)
                nc.tensor.matmul(ps_l[:], ones_s[:], gb_s[:],
                                 start=False, stop=True)
                lg = rt.tile([P, E], F32)
                nc.vector.tensor_copy(lg[:], ps_l[:])
                mx = rt.tile([P, E], F32)
                nc.vector.max(mx[:], lg[:])  # top-8 sorted desc; cols 0,1 = top-2
                sub = rt.tile([P, E], F32)
                nc.vector.tensor_scalar(sub[:], lg[:], mx[:, 0:1], None,
                                        op0=ALU.subtract)
                expl = rt.tile([P, E], F32)
                nc.scalar.activation(expl[:], sub[:], AFT.Exp)
                d21 = rt.tile([P, 1], F32)
                nc.vector.tensor_sub(d21[:], mx[:, 1:2], mx[:, 0:1])
                ed = rt.tile([P, 1], F32)
                nc.scalar.activation(ed[:], d21[:], AFT.Exp)
                den = rt.tile([P, 1], F32)
                nc.vector.tensor_scalar_add(den[:], ed[:], 1.0)
                rec = rt.tile([P, 1], F32)
                nc.vector.reciprocal(rec[:], den[:])
                msk = rt.tile([P, E], F32)
                nc.vector.tensor_scalar(msk[:], lg[:], mx[:, 1:2], None,
                                        op0=ALU.is_ge)
                cmb = rt.tile([P, E], F32)
                nc.vector.tensor_mul(cmb[:], expl[:], msk[:])
                nc.vector.tensor_scalar_mul(cmb[:], cmb[:], rec[:])
                ps_t = psr.tile([E, P], F32, tag="psr")
                nc.tensor.transpose(ps_t[:], cmb[:], id_s[:])
                nc.vector.tensor_copy(combT8[:, tok], ps_t[:])
            # f32r copy (SBUF->SBUF casting DMA rounds to f32r)
            nc.gpsimd.dma_start(combT8r[:], combT8[:])
            # broadcast comb row e across all 128 partitions
            # (stage row to partition 0 first: ISA ops need partition-0 base)
            for e in range(E):
                ct1 = rt.tile([1, T], F32, tag="ct1")
                nc.gpsimd.dma_start(ct1[:], combT8[e:e + 1, :])
                nc.gpsimd.partition_broadcast(comb_b[:, e, :], ct1[:])

            # ---- experts ----
            for e in range(E):
                b1_s = biasp.tile([P, NH], F32)
                nc.sync.dma_start(b1_s[:], b1t[e])
                for h in range(NH):
                    w1_s = w1p.tile([P, KD, P], F32R)
                    nc.sync.dma_start(w1_s[:], w1t[e, h])
                    ph = psh.tile([P, T], F32)
                    for kd in range(KD):
                        nc.tensor.matmul(ph[:], w1_s[:, kd, :], xTt[:, kd, :],
                                         start=(kd == 0), stop=(kd == KD - 1))
                    gt = gtmp.tile([P, T], F32)
                    nc.scalar.activation(gt[:], ph[:], AFT.Gelu,
                                         bias=b1_s[:, h:h + 1])
                    nc.vector.tensor_mul(geluT[:, h, :], gt[:],
                                         comb_b[:, e, :])
                for d in range(ND):
                    w2_s = w2p.tile([P, NH, P], F32R)
                    nc.sync.dma_start(w2_s[:], w2t[e, d])
                    po = pso.tile([P, T], F32)
                    for h in range(NH):
                        last = (h == NH - 1) and e != 0
                        nc.tensor.matmul(po[:], w2_s[:, h, :], geluT[:, h, :],
                                         start=(h == 0), stop=last)
                    if e == 0:
                        # bias term: sum_e comb[t,e]*b2[e,:] added once
                        nc.tensor.matmul(po[:], b2_s[:, d * P:(d + 1) * P],
                                         combT8r[:], start=False, stop=True)
                        nc.vector.tensor_copy(accT[:, d, :], po[:])
                    else:
                        nc.vector.tensor_add(accT[:, d, :], accT[:, d, :],
                                             po[:])
            for d in range(ND):
                nc.sync.dma_start(outT[d * P:(d + 1) * P, :], accT[:, d, :])

    nc.compile()
    return nc


def _get_nc():
    global _NC
    if _NC is None:
        _NC = _build()
    return _NC


def _prep_in_maps(x, gate_w, gate_b, w1, b1, w2, b2):
    f = np.float32
    x = np.asarray(x, f)
    gate_w = np.asarray(gate_w, f)
    gate_b = np.asarray(gate_b, f)
    w1 = np.asarray(w1, f)
    b1 = np.asarray(b1, f)
    w2 = np.asarray(w2, f)
    b2 = np.asarray(b2, f)

    xf = x.reshape(NTOK, D)
    gwt = np.ascontiguousarray(
        gate_w.reshape(KD, P, E).transpose(1, 0, 2))
    gbr = gate_b.reshape(1, E).copy()
    w1t = np.ascontiguousarray(
        w1.reshape(E, KD, P, NH, P).transpose(0, 3, 2, 1, 4))
    b1t = np.ascontiguousarray(b1.reshape(E, NH, P).transpose(0, 2, 1))
    w2t = np.ascontiguousarray(
        w2.reshape(E, NH, P, ND, P).transpose(0, 3, 2, 1, 4))
    ones = np.ones((1, P), f)
    ident = np.eye(P, dtype=f)

    shared = {"gwt": gwt, "gb": gbr, "w1t": w1t, "b1t": b1t, "w2t": w2t,
              "b2n": b2.copy(), "ones": ones, "ident": ident}
    in_maps = []
    for c in range(NCORES):
        xs = xf[c * T:(c + 1) * T]
        m = dict(shared)
        xt = np.ascontiguousarray(xs.T)
        m["xT"] = xt
        m["xTf"] = xt
        in_maps.append(m)
    return in_maps


def _assemble(results):
    outs = [np.asarray(results[c]["outT"]).T for c in range(NCORES)]
    return np.concatenate(outs, axis=0).reshape(2, 2048, D).astype(np.float32)


def run(inputs, trace=False):
    """Run the kernel; returns (output, exec_time_ns or None)."""
    in_maps = _prep_in_maps(**inputs)
    nc = _get_nc()
    res = bass_utils.run_bass_kernel_spmd(
        nc, in_maps, core_ids=list(range(NCORES)), trace=trace)
    return _assemble(res.results), res.exec_time_ns


def kernel(**inputs):
    out, _ = run(inputs, trace=False)
    return out


# revision 10
# speedup vs baseline: 1.0487x; 1.0487x over previous
"""MoE layer (E=8 experts, top-2, D=1024, H=4096, N=4096 tokens) on 8 TRN2
NeuronCores.

Strategy: data-parallel over tokens (512 tokens/core), experts replicated.
Each core computes the full MoE for its token shard entirely on device:
gate matmul + softmax-top2 routing, then the dense expert sum
   out = sum_e comb[:, e] * (gelu(x @ w1[e] + b1[e]) @ w2[e] + b2[e])
with comb[t, e] = 0 for experts outside token t's top-2 (so only the top-2
terms contribute, exactly matching the reference formulation).

All activations are kept transposed on device (feature dim on partitions,
tokens on the free axis) so both expert matmuls consume weights in their
natural layout and chain without transposes:
   hT[h, t]  = sum_d w1[d, h] * xT[d, t]        (lhsT = w1 tile, rhs = xT)
   outT[d, t] = sum_h w2[h, d] * geluT[h, t]     (lhsT = w2 tile, rhs = geluT)
Host does the pure layout work: shard + transpose x, pre-tile the weights,
transpose the returned outT shards back.

Matmuls run in fp32r (TF32-like, 1 PE cycle/row at N=512 -> ~78 TFLOP/s/core)
with fp32 PSUM accumulation.
"""

import numpy as np

import concourse.bass as bass  # noqa: F401  (bass types used via tile/bacc)
import concourse.mybir as mybir
import concourse.tile as tile
from concourse import bacc, bass_utils

F32 = mybir.dt.float32
F32R = mybir.dt.float32r
AFT = mybir.ActivationFunctionType
ALU = mybir.AluOpType

E = 8          # experts
D = 1024       # model dim
H = 4096       # expert hidden dim
P = 128        # partitions
NCORES = 8
NTOK = 4096    # total tokens (B*T = 2*2048)
T = NTOK // NCORES   # 512 tokens per core
KD = D // P    # 8 contraction chunks of D
NH = H // P    # 32 h tiles
ND = D // P    # 8 d tiles
TT = T // P    # 4 token tiles of 128

_NC = None  # cached compiled module


def _build():
    nc = bacc.Bacc("TRN2", target_bir_lowering=False, debug=False,
                   num_devices=NCORES)
    xT = nc.dram_tensor("xT", [D, T], F32R, kind="ExternalInput").ap()
    xTf = nc.dram_tensor("xTf", [D, T], F32, kind="ExternalInput").ap()
    gwt = nc.dram_tensor("gwt", [P, KD, E], F32, kind="ExternalInput").ap()
    gb = nc.dram_tensor("gb", [1, E], F32, kind="ExternalInput").ap()
    w1t = nc.dram_tensor("w1t", [E, NH, P, KD, P], F32R,
                         kind="ExternalInput").ap()
    b1t = nc.dram_tensor("b1t", [E, P, NH], F32, kind="ExternalInput").ap()
    w2t = nc.dram_tensor("w2t", [E, ND, P, NH, P], F32R,
                         kind="ExternalInput").ap()
    b2n = nc.dram_tensor("b2n", [E, D], F32R, kind="ExternalInput").ap()
    ones = nc.dram_tensor("ones", [1, P], F32, kind="ExternalInput").ap()
    ident = nc.dram_tensor("ident", [P, P], F32, kind="ExternalInput").ap()
    outT = nc.dram_tensor("outT", [D, T], F32, kind="ExternalOutput").ap()

    with tile.TileContext(nc) as tc:
        with (
            tc.tile_pool(name="const", bufs=1) as cpool,
            tc.tile_pool(name="w1p", bufs=3) as w1p,
            tc.tile_pool(name="w2p", bufs=2) as w2p,
            tc.tile_pool(name="bias", bufs=2) as biasp,
            tc.tile_pool(name="rt", bufs=2) as rt,
            tc.tile_pool(name="xfp", bufs=3) as xfp,
            tc.tile_pool(name="gtmp", bufs=4) as gtmp,
            tc.tile_pool(name="psh", bufs=3, space="PSUM") as psh,
            tc.tile_pool(name="pso", bufs=3, space="PSUM") as pso,
            tc.tile_pool(name="psr", bufs=2, space="PSUM") as psr,
        ):
            # ---- persistent SBUF ----
            gwt_s = cpool.tile([P, KD, E], F32)
            nc.sync.dma_start(gwt_s[:], gwt[:])
            gb_s = cpool.tile([1, E], F32)
            nc.sync.dma_start(gb_s[:], gb[:])
            ones_s = cpool.tile([1, P], F32)
            nc.sync.dma_start(ones_s[:], ones[:])
            id_s = cpool.tile([P, P], F32)
            nc.sync.dma_start(id_s[:], ident[:])
            b2_s = cpool.tile([E, D], F32R)
            nc.sync.dma_start(b2_s[:], b2n[:])
            combT8 = cpool.tile([E, T], F32)    # comb.T on partitions 0..7
            combT8r = cpool.tile([E, T], F32R)  # f32r copy for the b2 matmul
            comb_b = cpool.tile([P, E, T], F32)
            accT = cpool.tile([P, ND, T], F32)
            geluT = cpool.tile([P, NH, T], F32R)

            # ---- routing: logits -> top-2 -> comb weights ----
            for tt in range(TT):
                tok = slice(tt * P, (tt + 1) * P)
                ps_l = psr.tile([P, E], F32, tag="psr")
                for kd in range(KD):
                    xf = xfp.tile([P, P], F32, tag="xf")
                    nc.sync.dma_start(xf[:], xTf[kd * P:(kd + 1) * P, tok])
                    nc.tensor.matmul(ps_l[:], xf[:], gwt_s[:, kd, :],
                                     start=(kd == 0), stop=False)
                nc.tensor.matmul(ps_l[:], ones_s[:], gb_s[:],
                                 start=False, stop=True)
                lg = rt.tile([P, E], F32)
                nc.vector.tensor_copy(lg[:], ps_l[:])
                mx = rt.tile([P, E], F32)
                nc.vector.max(mx[:], lg[:])  # top-8 sorted desc; cols 0,1 = top-2
                sub = rt.tile([P, E], F32)
                nc.vector.tensor_scalar(sub[:], lg[:], mx[:, 0:1], None,
                                        op0=ALU.subtract)
                expl = rt.tile([P, E], F32)
                nc.scalar.activation(expl[:], sub[:], AFT.Exp)
                d21 = rt.tile([P, 1], F32)
                nc.vector.tensor_sub(d21[:], mx[:, 1:2], mx[:, 0:1])
                ed = rt.tile([P, 1], F32)
                nc.scalar.activation(ed[:], d21[:], AFT.Exp)
                den = rt.tile([P, 1], F32)
                nc.vector.tensor_scalar_add(den[:], ed[:], 1.0)
                rec = rt.tile([P, 1], F32)
                nc.vector.reciprocal(rec[:], den[:])
                msk = rt.tile([P, E], F32)
                nc.vector.tensor_scalar(msk[:], lg[:], mx[:, 1:2], None,
                                        op0=ALU.is_ge)
                cmb = rt.tile([P, E], F32)
                nc.vector.tensor_mul(cmb[:], expl[:], msk[:])
                nc.vector.tensor_scalar_mul(cmb[:], cmb[:], rec[:])
                ps_t = psr.tile([E, P], F32, tag="psr")
                nc.tensor.transpose(ps_t[:], cmb[:], id_s[:])
                nc.vector.tensor_copy(combT8[:, tok], ps_t[:])
            # f32r copy (SBUF->SBUF casting DMA rounds to f32r)
            nc.gpsimd.dma_start(combT8r[:], combT8[:])
            # broadcast comb row e across all 128 partitions
            # (stage row to partition 0 first: ISA ops need partition-0 base)
            for e in range(E):
                ct1 = rt.tile([1, T], F32, tag="ct1")
                nc.gpsimd.dma_start(ct1[:], combT8[e:e + 1, :])
                nc.gpsimd.partition_broadcast(comb_b[:, e, :], ct1[:])

            # xT (f32r, rounded) for the expert matmuls; loaded during routing
            xTt = cpool.tile([P, KD, T], F32R)
            for kd in range(KD):
                nc.sync.dma_start(xTt[:, kd, :], xT[kd * P:(kd + 1) * P, :])

            # ---- experts ----
            for e in range(E):
                b1_s = biasp.tile([P, NH], F32)
                nc.sync.dma_start(b1_s[:], b1t[e])
                for h in range(NH):
                    w1_s = w1p.tile([P, KD, P], F32R)
                    nc.sync.dma_start(w1_s[:, 0:KD // 2, :],
                                      w1t[e, h, :, 0:KD // 2, :])
                    nc.sync.dma_start(w1_s[:, KD // 2:, :],
                                      w1t[e, h, :, KD // 2:, :])
                    ph = psh.tile([P, T], F32)
                    for kd in range(KD):
                        nc.tensor.matmul(ph[:], w1_s[:, kd, :], xTt[:, kd, :],
                                         start=(kd == 0), stop=(kd == KD - 1))
                    gt = gtmp.tile([P, T], F32)
                    nc.scalar.activation(gt[:], ph[:], AFT.Gelu,
                                         bias=b1_s[:, h:h + 1])
                    nc.vector.tensor_mul(geluT[:, h, :], gt[:],
                                         comb_b[:, e, :])
                for d in range(ND):
                    w2_s = w2p.tile([P, NH, P], F32R)
                    nc.sync.dma_start(w2_s[:, 0:NH // 2, :],
                                      w2t[e, d, :, 0:NH // 2, :])
                    nc.sync.dma_start(w2_s[:, NH // 2:, :],
                                      w2t[e, d, :, NH // 2:, :])
                    po = pso.tile([P, T], F32)
                    for h in range(NH):
                        last = (h == NH - 1) and e != 0
                        nc.tensor.matmul(po[:], w2_s[:, h, :], geluT[:, h, :],
                                         start=(h == 0), stop=last)
                    if e == 0:
                        # bias term: sum_e comb[t,e]*b2[e,:] added once
                        nc.tensor.matmul(po[:], b2_s[:, d * P:(d + 1) * P],
                                         combT8r[:], start=False, stop=True)
                        nc.vector.tensor_copy(accT[:, d, :], po[:])
                    else:
                        nc.vector.tensor_add(accT[:, d, :], accT[:, d, :],
                                             po[:])
            for d in range(ND):
                nc.sync.dma_start(outT[d * P:(d + 1) * P, :], accT[:, d, :])

    nc.compile()
    return nc


def _get_nc():
    global _NC
    if _NC is None:
        _NC = _build()
    return _NC


def _prep_in_maps(x, gate_w, gate_b, w1, b1, w2, b2):
    f = np.float32
    x = np.asarray(x, f)
    gate_w = np.asarray(gate_w, f)
    gate_b = np.asarray(gate_b, f)
    w1 = np.asarray(w1, f)
    b1 = np.asarray(b1, f)
    w2 = np.asarray(w2, f)
    b2 = np.asarray(b2, f)

    xf = x.reshape(NTOK, D)
    gwt = np.ascontiguousarray(
        gate_w.reshape(KD, P, E).transpose(1, 0, 2))
    gbr = gate_b.reshape(1, E).copy()
    w1t = np.ascontiguousarray(
        w1.reshape(E, KD, P, NH, P).transpose(0, 3, 2, 1, 4))
    b1t = np.ascontiguousarray(b1.reshape(E, NH, P).transpose(0, 2, 1))
    w2t = np.ascontiguousarray(
        w2.reshape(E, NH, P, ND, P).transpose(0, 3, 2, 1, 4))
    ones = np.ones((1, P), f)
    ident = np.eye(P, dtype=f)

    shared = {"gwt": gwt, "gb": gbr, "w1t": w1t, "b1t": b1t, "w2t": w2t,
              "b2n": b2.copy(), "ones": ones, "ident": ident}
    in_maps = []
    for c in range(NCORES):
        xs = xf[c * T:(c + 1) * T]
        m = dict(shared)
        xt = np.ascontiguousarray(xs.T)
        m["xT"] = xt
        m["xTf"] = xt
        in_maps.append(m)
    return in_maps


def _assemble(results):
    outs = [np.asarray(results[c]["outT"]).T for c in range(NCORES)]
    return np.concatenate(outs, axis=0).reshape(2, 2048, D).astype(np.float32)


def run(inputs, trace=False):
    """Run the kernel; returns (output, exec_time_ns or None)."""
    in_maps = _prep_in_maps(**inputs)
    nc = _get_nc()
    res = bass_utils.run_bass_kernel_spmd(
        nc, in_maps, core_ids=list(range(NCORES)), trace=trace)
    return _assemble(res.results), res.exec_time_ns


def kernel(**inputs):
    out, _ = run(inputs, trace=False)
    return out


# revision 12
# speedup vs baseline: 1.0593x; 1.0101x over previous
"""MoE layer (E=8 experts, top-2, D=1024, H=4096, N=4096 tokens) on 8 TRN2
NeuronCores.

Strategy: data-parallel over tokens (512 tokens/core), experts replicated.
Each core computes the full MoE for its token shard entirely on device:
gate matmul + softmax-top2 routing, then the dense expert sum
   out = sum_e comb[:, e] * (gelu(x @ w1[e] + b1[e]) @ w2[e] + b2[e])
with comb[t, e] = 0 for experts outside token t's top-2 (so only the top-2
terms contribute, exactly matching the reference formulation).

All activations are kept transposed on device (feature dim on partitions,
tokens on the free axis) so both expert matmuls consume weights in their
natural layout and chain without transposes:
   hT[h, t]  = sum_d w1[d, h] * xT[d, t]        (lhsT = w1 tile, rhs = xT)
   outT[d, t] = sum_h w2[h, d] * geluT[h, t]     (lhsT = w2 tile, rhs = geluT)
Host does the pure layout work: shard + transpose x, pre-tile the weights,
transpose the returned outT shards back.

Matmuls run in fp32r (TF32-like, 1 PE cycle/row at N=512 -> ~78 TFLOP/s/core)
with fp32 PSUM accumulation.
"""

import numpy as np

import concourse.bass as bass  # noqa: F401  (bass types used via tile/bacc)
import concourse.mybir as mybir
import concourse.tile as tile
from concourse import bacc, bass_utils

F32 = mybir.dt.float32
F32R = mybir.dt.float32r
AFT = mybir.ActivationFunctionType
ALU = mybir.AluOpType

E = 8          # experts
D = 1024       # model dim
H = 4096       # expert hidden dim
P = 128        # partitions
NCORES = 8
NTOK = 4096    # total tokens (B*T = 2*2048)
T = NTOK // NCORES   # 512 tokens per core
KD = D // P    # 8 contraction chunks of D
NH = H // P    # 32 h tiles
ND = D // P    # 8 d tiles
TT = T // P    # 4 token tiles of 128

_NC = None  # cached compiled module


def _build():
    nc = bacc.Bacc("TRN2", target_bir_lowering=False, debug=False,
                   num_devices=NCORES)
    xT = nc.dram_tensor("xT", [D, T], F32R, kind="ExternalInput").ap()
    xTf = nc.dram_tensor("xTf", [D, T], F32, kind="ExternalInput").ap()
    gwt = nc.dram_tensor("gwt", [P, KD, E], F32, kind="ExternalInput").ap()
    gb = nc.dram_tensor("gb", [1, E], F32, kind="ExternalInput").ap()
    w1t = nc.dram_tensor("w1t", [E, NH, P, KD, P], F32R,
                         kind="ExternalInput").ap()
    b1t = nc.dram_tensor("b1t", [E, P, NH], F32, kind="ExternalInput").ap()
    w2t = nc.dram_tensor("w2t", [E, ND, P, NH, P], F32R,
                         kind="ExternalInput").ap()
    b2n = nc.dram_tensor("b2n", [E, D], F32R, kind="ExternalInput").ap()
    ones = nc.dram_tensor("ones", [1, P], F32, kind="ExternalInput").ap()
    ident = nc.dram_tensor("ident", [P, P], F32, kind="ExternalInput").ap()
    outT = nc.dram_tensor("outT", [D, T], F32, kind="ExternalOutput").ap()

    with tile.TileContext(nc) as tc:
        with (
            tc.tile_pool(name="const", bufs=1) as cpool,
            tc.tile_pool(name="w1p", bufs=3) as w1p,
            tc.tile_pool(name="w2p", bufs=2) as w2p,
            tc.tile_pool(name="bias", bufs=2) as biasp,
            tc.tile_pool(name="rt", bufs=2) as rt,
            tc.tile_pool(name="xfp", bufs=3) as xfp,
            tc.tile_pool(name="gtmp", bufs=8) as gtmp,
            tc.tile_pool(name="psh", bufs=3, space="PSUM") as psh,
            tc.tile_pool(name="pso", bufs=3, space="PSUM") as pso,
            tc.tile_pool(name="psr", bufs=2, space="PSUM") as psr,
        ):
            # ---- persistent SBUF ----
            gwt_s = cpool.tile([P, KD, E], F32)
            nc.sync.dma_start(gwt_s[:], gwt[:])
            gb_s = cpool.tile([1, E], F32)
            nc.sync.dma_start(gb_s[:], gb[:])
            ones_s = cpool.tile([1, P], F32)
            nc.sync.dma_start(ones_s[:], ones[:])
            id_s = cpool.tile([P, P], F32)
            nc.sync.dma_start(id_s[:], ident[:])
            b2_s = cpool.tile([E, D], F32R)
            nc.sync.dma_start(b2_s[:], b2n[:])
            combT8 = cpool.tile([E, T], F32)    # comb.T on partitions 0..7
            combT8r = cpool.tile([E, T], F32R)  # f32r copy for the b2 matmul
            comb_b = cpool.tile([P, E, T], F32)
            accT = cpool.tile([P, ND, T], F32)
            geluT = cpool.tile([P, NH, T], F32R)

            # ---- routing, emitted as closures and interleaved into expert
            # 0's mm1 loop so the DVE/ACT round-trips overlap PE matmuls ----
            cmb_tiles = {}

            def routing_part1(tt):
                tok = slice(tt * P, (tt + 1) * P)
                ps_l = psr.tile([P, E], F32, tag="psr")
                for kd in range(KD):
                    xf = xfp.tile([P, P], F32, tag="xf")
                    nc.sync.dma_start(xf[:], xTf[kd * P:(kd + 1) * P, tok])
                    nc.tensor.matmul(ps_l[:], xf[:], gwt_s[:, kd, :],
                                     start=(kd == 0), stop=False)
                nc.tensor.matmul(ps_l[:], ones_s[:], gb_s[:],
                                 start=False, stop=True)
                lg = rt.tile([P, E], F32)
                nc.vector.tensor_copy(lg[:], ps_l[:])
                mx = rt.tile([P, E], F32)
                nc.vector.max(mx[:], lg[:])  # top-8 sorted desc; 0,1 = top-2
                sub = rt.tile([P, E], F32)
                nc.vector.tensor_scalar(sub[:], lg[:], mx[:, 0:1], None,
                                        op0=ALU.subtract)
                expl = rt.tile([P, E], F32)
                nc.scalar.activation(expl[:], sub[:], AFT.Exp)
                d21 = rt.tile([P, 1], F32)
                nc.vector.tensor_sub(d21[:], mx[:, 1:2], mx[:, 0:1])
                ed = rt.tile([P, 1], F32)
                nc.scalar.activation(ed[:], d21[:], AFT.Exp)
                den = rt.tile([P, 1], F32)
                nc.vector.tensor_scalar_add(den[:], ed[:], 1.0)
                rec = rt.tile([P, 1], F32)
                nc.vector.reciprocal(rec[:], den[:])
                msk = rt.tile([P, E], F32)
                nc.vector.tensor_scalar(msk[:], lg[:], mx[:, 1:2], None,
                                        op0=ALU.is_ge)
                cmb = rt.tile([P, E], F32)
                nc.vector.tensor_mul(cmb[:], expl[:], msk[:])
                nc.vector.tensor_scalar_mul(cmb[:], cmb[:], rec[:])
                cmb_tiles[tt] = cmb

            def routing_part2(tt):
                tok = slice(tt * P, (tt + 1) * P)
                cmb = cmb_tiles.pop(tt)
                ps_t = psr.tile([E, P], F32, tag="psr")
                nc.tensor.transpose(ps_t[:], cmb[:], id_s[:])
                nc.vector.tensor_copy(combT8[:, tok], ps_t[:])

            def routing_finish():
                # f32r copy (SBUF->SBUF casting DMA rounds to f32r)
                nc.gpsimd.dma_start(combT8r[:], combT8[:])
                # broadcast comb row e across all 128 partitions (stage row
                # to partition 0 first: ISA ops need partition-0 base)
                for e in range(E):
                    ct1 = rt.tile([1, T], F32, tag="ct1")
                    nc.gpsimd.dma_start(ct1[:], combT8[e:e + 1, :])
                    nc.gpsimd.partition_broadcast(comb_b[:, e, :], ct1[:])

            # xT (f32r, rounded) for the expert matmuls
            xTt = cpool.tile([P, KD, T], F32R)
            for kd in range(KD):
                nc.sync.dma_start(xTt[:, kd, :], xT[kd * P:(kd + 1) * P, :])

            # ---- experts ----
            deferred = []
            for e in range(E):
                b1_s = biasp.tile([P, NH], F32)
                nc.sync.dma_start(b1_s[:], b1t[e])
                for h in range(NH):
                    w1_s = w1p.tile([P, KD, P], F32R)
                    nc.sync.dma_start(w1_s[:, 0:KD // 2, :],
                                      w1t[e, h, :, 0:KD // 2, :])
                    nc.sync.dma_start(w1_s[:, KD // 2:, :],
                                      w1t[e, h, :, KD // 2:, :])
                    ph = psh.tile([P, T], F32)
                    for kd in range(KD):
                        nc.tensor.matmul(ph[:], w1_s[:, kd, :], xTt[:, kd, :],
                                         start=(kd == 0), stop=(kd == KD - 1))
                    gt = gtmp.tile([P, T], F32)
                    nc.scalar.activation(gt[:], ph[:], AFT.Gelu,
                                         bias=b1_s[:, h:h + 1])
                    if e == 0 and h <= TT + 1:
                        # routing runs under the first mm1 tiles; comb_b is
                        # not written yet, so defer these tiles' scale-muls
                        # until routing_finish (reads must be emitted after
                        # the write for Tile to see the RAW dependency)
                        if h < TT:
                            routing_part1(h)
                        if 1 <= h <= TT:
                            routing_part2(h - 1)
                        deferred.append((h, gt))
                        if h == TT + 1:
                            routing_finish()
                            for hh, g in deferred:
                                nc.vector.tensor_mul(geluT[:, hh, :], g[:],
                                                     comb_b[:, e, :])
                            deferred.clear()
                    else:
                        nc.vector.tensor_mul(geluT[:, h, :], gt[:],
                                             comb_b[:, e, :])
                for d in range(ND):
                    w2_s = w2p.tile([P, NH, P], F32R)
                    nc.sync.dma_start(w2_s[:, 0:NH // 2, :],
                                      w2t[e, d, :, 0:NH // 2, :])
                    nc.sync.dma_start(w2_s[:, NH // 2:, :],
                                      w2t[e, d, :, NH // 2:, :])
                    po = pso.tile([P, T], F32)
                    for h in range(NH):
                        last = (h == NH - 1) and e != 0
                        nc.tensor.matmul(po[:], w2_s[:, h, :], geluT[:, h, :],
                                         start=(h == 0), stop=last)
                    if e == 0:
                        # bias term: sum_e comb[t,e]*b2[e,:] added once
                        nc.tensor.matmul(po[:], b2_s[:, d * P:(d + 1) * P],
                                         combT8r[:], start=False, stop=True)
                        nc.vector.tensor_copy(accT[:, d, :], po[:])
                    else:
                        nc.vector.tensor_add(accT[:, d, :], accT[:, d, :],
                                             po[:])
            for d in range(ND):
                nc.sync.dma_start(outT[d * P:(d + 1) * P, :], accT[:, d, :])

    nc.compile()
    return nc


def _get_nc():
    global _NC
    if _NC is None:
        _NC = _build()
    return _NC


def _prep_in_maps(x, gate_w, gate_b, w1, b1, w2, b2):
    f = np.float32
    x = np.asarray(x, f)
    gate_w = np.asarray(gate_w, f)
    gate_b = np.asarray(gate_b, f)
    w1 = np.asarray(w1, f)
    b1 = np.asarray(b1, f)
    w2 = np.asarray(w2, f)
    b2 = np.asarray(b2, f)

    xf = x.reshape(NTOK, D)
    gwt = np.ascontiguousarray(
        gate_w.reshape(KD, P, E).transpose(1, 0, 2))
    gbr = gate_b.reshape(1, E).copy()
    w1t = np.ascontiguousarray(
        w1.reshape(E, KD, P, NH, P).transpose(0, 3, 2, 1, 4))
    b1t = np.ascontiguousarray(b1.reshape(E, NH, P).transpose(0, 2, 1))
    w2t = np.ascontiguousarray(
        w2.reshape(E, NH, P, ND, P).transpose(0, 3, 2, 1, 4))
    ones = np.ones((1, P), f)
    ident = np.eye(P, dtype=f)

    shared = {"gwt": gwt, "gb": gbr, "w1t": w1t, "b1t": b1t, "w2t": w2t,
              "b2n": b2.copy(), "ones": ones, "ident": ident}
    in_maps = []
    for c in range(NCORES):
        xs = xf[c * T:(c + 1) * T]
        m = dict(shared)
        xt = np.ascontiguousarray(xs.T)
        m["xT"] = xt
        m["xTf"] = xt
        in_maps.append(m)
    return in_maps


def _assemble(results):
    outs = [np.asarray(results[c]["outT"]).T for c in range(NCORES)]
    return np.concatenate(outs, axis=0).reshape(2, 2048, D).astype(np.float32)


def run(inputs, trace=False):
    """Run the kernel; returns (output, exec_time_ns or None)."""
    in_maps = _prep_in_maps(**inputs)
    nc = _get_nc()
    res = bass_utils.run_bass_kernel_spmd(
        nc, in_maps, core_ids=list(range(NCORES)), trace=trace)
    return _assemble(res.results), res.exec_time_ns


def kernel(**inputs):
    out, _ = run(inputs, trace=False)
    return out


# revision 13
# speedup vs baseline: 1.1007x; 1.0391x over previous
"""MoE layer (E=8 experts, top-2, D=1024, H=4096, N=4096 tokens) on 8 TRN2
NeuronCores.

Strategy: data-parallel over tokens (512 tokens/core), experts replicated.
Each core computes the full MoE for its token shard entirely on device:
gate matmul + softmax-top2 routing, then the dense expert sum
   out = sum_e comb[:, e] * (gelu(x @ w1[e] + b1[e]) @ w2[e] + b2[e])
with comb[t, e] = 0 for experts outside token t's top-2 (so only the top-2
terms contribute, exactly matching the reference formulation).

All activations are kept transposed on device (feature dim on partitions,
tokens on the free axis) so both expert matmuls consume weights in their
natural layout and chain without transposes:
   hT[h, t]  = sum_d w1[d, h] * xT[d, t]        (lhsT = w1 tile, rhs = xT)
   outT[d, t] = sum_h w2[h, d] * geluT[h, t]     (lhsT = w2 tile, rhs = geluT)
Host does the pure layout work: shard + transpose x, pre-tile the weights,
transpose the returned outT shards back.

Matmuls run in fp32r (TF32-like, 1 PE cycle/row at N=512 -> ~78 TFLOP/s/core)
with fp32 PSUM accumulation.
"""

import numpy as np

import concourse.bass as bass  # noqa: F401  (bass types used via tile/bacc)
import concourse.mybir as mybir
import concourse.tile as tile
from concourse import bacc, bass_utils

F32 = mybir.dt.float32
F32R = mybir.dt.float32r
AFT = mybir.ActivationFunctionType
ALU = mybir.AluOpType

E = 8          # experts
D = 1024       # model dim
H = 4096       # expert hidden dim
P = 128        # partitions
NCORES = 8
NTOK = 4096    # total tokens (B*T = 2*2048)
T = NTOK // NCORES   # 512 tokens per core
KD = D // P    # 8 contraction chunks of D
NH = H // P    # 32 h tiles
ND = D // P    # 8 d tiles
TT = T // P    # 4 token tiles of 128

_NC = None  # cached compiled module


def _build():
    nc = bacc.Bacc("TRN2", target_bir_lowering=False, debug=False,
                   num_devices=NCORES)
    xT = nc.dram_tensor("xT", [D, T], F32R, kind="ExternalInput").ap()
    xTf = nc.dram_tensor("xTf", [D, T], F32, kind="ExternalInput").ap()
    gwt = nc.dram_tensor("gwt", [P, KD, E], F32, kind="ExternalInput").ap()
    gb = nc.dram_tensor("gb", [1, E], F32, kind="ExternalInput").ap()
    w1t = nc.dram_tensor("w1t", [E, NH, P, KD, P], F32R,
                         kind="ExternalInput").ap()
    b1t = nc.dram_tensor("b1t", [E, P, NH], F32, kind="ExternalInput").ap()
    w2t = nc.dram_tensor("w2t", [E, ND, P, NH, P], F32R,
                         kind="ExternalInput").ap()
    b2n = nc.dram_tensor("b2n", [E, D], F32R, kind="ExternalInput").ap()
    ones = nc.dram_tensor("ones", [1, P], F32, kind="ExternalInput").ap()
    ident = nc.dram_tensor("ident", [P, P], F32, kind="ExternalInput").ap()
    outT = nc.dram_tensor("outT", [D, T], F32, kind="ExternalOutput").ap()

    with tile.TileContext(nc) as tc:
        with (
            tc.tile_pool(name="const", bufs=1) as cpool,
            tc.tile_pool(name="w1p", bufs=4) as w1p,
            tc.tile_pool(name="w2p", bufs=2) as w2p,
            tc.tile_pool(name="bias", bufs=2) as biasp,
            tc.tile_pool(name="rt", bufs=2) as rt,
            tc.tile_pool(name="xfp", bufs=3) as xfp,
            tc.tile_pool(name="gtmp", bufs=8) as gtmp,
            tc.tile_pool(name="psh", bufs=4, space="PSUM") as psh,
            tc.tile_pool(name="pso", bufs=4, space="PSUM") as pso,
        ):
            # ---- persistent SBUF ----
            gwt_s = cpool.tile([P, KD, E], F32)
            nc.sync.dma_start(gwt_s[:], gwt[:])
            gb_s = cpool.tile([1, E], F32)
            nc.sync.dma_start(gb_s[:], gb[:])
            ones_s = cpool.tile([1, P], F32)
            nc.sync.dma_start(ones_s[:], ones[:])
            id_s = cpool.tile([P, P], F32)
            nc.sync.dma_start(id_s[:], ident[:])
            b2_s = cpool.tile([E, D], F32R)
            nc.sync.dma_start(b2_s[:], b2n[:])
            combT8 = cpool.tile([E, T], F32)    # comb.T on partitions 0..7
            combT8r = cpool.tile([E, T], F32R)  # f32r copy for the b2 matmul
            comb_b = cpool.tile([P, E, T], F32)
            accT = cpool.tile([P, ND, T], F32)
            geluT = cpool.tile([P, NH, T], F32R)

            # ---- routing, emitted as closures and interleaved into expert
            # 0's mm1 loop so the DVE/ACT round-trips overlap PE matmuls ----
            cmb_tiles = {}

            def routing_part1(tt):
                tok = slice(tt * P, (tt + 1) * P)
                ps_l = pso.tile([P, E], F32, tag="po")
                for kd in range(KD):
                    xf = xfp.tile([P, P], F32, tag="xf")
                    nc.sync.dma_start(xf[:], xTf[kd * P:(kd + 1) * P, tok])
                    nc.tensor.matmul(ps_l[:], xf[:], gwt_s[:, kd, :],
                                     start=(kd == 0), stop=False)
                nc.tensor.matmul(ps_l[:], ones_s[:], gb_s[:],
                                 start=False, stop=True)
                lg = rt.tile([P, E], F32)
                nc.vector.tensor_copy(lg[:], ps_l[:])
                mx = rt.tile([P, E], F32)
                nc.vector.max(mx[:], lg[:])  # top-8 sorted desc; 0,1 = top-2
                sub = rt.tile([P, E], F32)
                nc.vector.tensor_scalar(sub[:], lg[:], mx[:, 0:1], None,
                                        op0=ALU.subtract)
                expl = rt.tile([P, E], F32)
                nc.scalar.activation(expl[:], sub[:], AFT.Exp)
                d21 = rt.tile([P, 1], F32)
                nc.vector.tensor_sub(d21[:], mx[:, 1:2], mx[:, 0:1])
                ed = rt.tile([P, 1], F32)
                nc.scalar.activation(ed[:], d21[:], AFT.Exp)
                den = rt.tile([P, 1], F32)
                nc.vector.tensor_scalar_add(den[:], ed[:], 1.0)
                rec = rt.tile([P, 1], F32)
                nc.vector.reciprocal(rec[:], den[:])
                msk = rt.tile([P, E], F32)
                nc.vector.tensor_scalar(msk[:], lg[:], mx[:, 1:2], None,
                                        op0=ALU.is_ge)
                cmb = rt.tile([P, E], F32)
                nc.vector.tensor_mul(cmb[:], expl[:], msk[:])
                nc.vector.tensor_scalar_mul(cmb[:], cmb[:], rec[:])
                cmb_tiles[tt] = cmb

            def routing_part2(tt):
                tok = slice(tt * P, (tt + 1) * P)
                cmb = cmb_tiles.pop(tt)
                ps_t = pso.tile([E, P], F32, tag="po")
                nc.tensor.transpose(ps_t[:], cmb[:], id_s[:])
                nc.vector.tensor_copy(combT8[:, tok], ps_t[:])

            def routing_finish():
                # f32r copy (SBUF->SBUF casting DMA rounds to f32r)
                nc.gpsimd.dma_start(combT8r[:], combT8[:])
                # broadcast comb row e across all 128 partitions (stage row
                # to partition 0 first: ISA ops need partition-0 base)
                for e in range(E):
                    ct1 = rt.tile([1, T], F32, tag="ct1")
                    nc.gpsimd.dma_start(ct1[:], combT8[e:e + 1, :])
                    nc.gpsimd.partition_broadcast(comb_b[:, e, :], ct1[:])

            # xT (f32r, rounded) for the expert matmuls
            xTt = cpool.tile([P, KD, T], F32R)
            for kd in range(KD):
                nc.sync.dma_start(xTt[:, kd, :], xT[kd * P:(kd + 1) * P, :])

            # ---- experts ----
            deferred = []
            for e in range(E):
                b1_s = biasp.tile([P, NH], F32)
                nc.sync.dma_start(b1_s[:], b1t[e])
                for h in range(NH):
                    w1_s = w1p.tile([P, KD, P], F32R)
                    nc.sync.dma_start(w1_s[:, 0:KD // 2, :],
                                      w1t[e, h, :, 0:KD // 2, :])
                    nc.sync.dma_start(w1_s[:, KD // 2:, :],
                                      w1t[e, h, :, KD // 2:, :])
                    ph = psh.tile([P, T], F32)
                    for kd in range(KD):
                        nc.tensor.matmul(ph[:], w1_s[:, kd, :], xTt[:, kd, :],
                                         start=(kd == 0), stop=(kd == KD - 1))
                    gt = gtmp.tile([P, T], F32)
                    nc.scalar.activation(gt[:], ph[:], AFT.Gelu,
                                         bias=b1_s[:, h:h + 1])
                    if e == 0 and h <= TT + 1:
                        # routing runs under the first mm1 tiles; comb_b is
                        # not written yet, so defer these tiles' scale-muls
                        # until routing_finish (reads must be emitted after
                        # the write for Tile to see the RAW dependency)
                        if h < TT:
                            routing_part1(h)
                        if 1 <= h <= TT:
                            routing_part2(h - 1)
                        deferred.append((h, gt))
                        if h == TT + 1:
                            routing_finish()
                            for hh, g in deferred:
                                nc.vector.tensor_mul(geluT[:, hh, :], g[:],
                                                     comb_b[:, e, :])
                            deferred.clear()
                    else:
                        nc.vector.tensor_mul(geluT[:, h, :], gt[:],
                                             comb_b[:, e, :])
                for d in range(ND):
                    w2_s = w2p.tile([P, NH, P], F32R)
                    for q in range(4):
                        nc.sync.dma_start(
                            w2_s[:, q * NH // 4:(q + 1) * NH // 4, :],
                            w2t[e, d, :, q * NH // 4:(q + 1) * NH // 4, :])
                    po = pso.tile([P, T], F32, tag="po")
                    for h in range(NH):
                        last = (h == NH - 1) and e != 0
                        nc.tensor.matmul(po[:], w2_s[:, h, :], geluT[:, h, :],
                                         start=(h == 0), stop=last)
                    if e == 0:
                        # bias term: sum_e comb[t,e]*b2[e,:] added once
                        nc.tensor.matmul(po[:], b2_s[:, d * P:(d + 1) * P],
                                         combT8r[:], start=False, stop=True)
                        nc.vector.tensor_copy(accT[:, d, :], po[:])
                    else:
                        nc.vector.tensor_add(accT[:, d, :], accT[:, d, :],
                                             po[:])
            for d in range(ND):
                nc.sync.dma_start(outT[d * P:(d + 1) * P, :], accT[:, d, :])

    nc.compile()
    return nc


def _get_nc():
    global _NC
    if _NC is None:
        _NC = _build()
    return _NC


def _prep_in_maps(x, gate_w, gate_b, w1, b1, w2, b2):
    f = np.float32
    x = np.asarray(x, f)
    gate_w = np.asarray(gate_w, f)
    gate_b = np.asarray(gate_b, f)
    w1 = np.asarray(w1, f)
    b1 = np.asarray(b1, f)
    w2 = np.asarray(w2, f)
    b2 = np.asarray(b2, f)

    xf = x.reshape(NTOK, D)
    gwt = np.ascontiguousarray(
        gate_w.reshape(KD, P, E).transpose(1, 0, 2))
    gbr = gate_b.reshape(1, E).copy()
    w1t = np.ascontiguousarray(
        w1.reshape(E, KD, P, NH, P).transpose(0, 3, 2, 1, 4))
    b1t = np.ascontiguousarray(b1.reshape(E, NH, P).transpose(0, 2, 1))
    w2t = np.ascontiguousarray(
        w2.reshape(E, NH, P, ND, P).transpose(0, 3, 2, 1, 4))
    ones = np.ones((1, P), f)
    ident = np.eye(P, dtype=f)

    shared = {"gwt": gwt, "gb": gbr, "w1t": w1t, "b1t": b1t, "w2t": w2t,
              "b2n": b2.copy(), "ones": ones, "ident": ident}
    in_maps = []
    for c in range(NCORES):
        xs = xf[c * T:(c + 1) * T]
        m = dict(shared)
        xt = np.ascontiguousarray(xs.T)
        m["xT"] = xt
        m["xTf"] = xt
        in_maps.append(m)
    return in_maps


def _assemble(results):
    outs = [np.asarray(results[c]["outT"]).T for c in range(NCORES)]
    return np.concatenate(outs, axis=0).reshape(2, 2048, D).astype(np.float32)


def run(inputs, trace=False):
    """Run the kernel; returns (output, exec_time_ns or None)."""
    in_maps = _prep_in_maps(**inputs)
    nc = _get_nc()
    res = bass_utils.run_bass_kernel_spmd(
        nc, in_maps, core_ids=list(range(NCORES)), trace=trace)
    return _assemble(res.results), res.exec_time_ns


def kernel(**inputs):
    out, _ = run(inputs, trace=False)
    return out
